# revision 2
# baseline (speedup 1.0000x reference)
"""LocalContrastEnhancement v4: one concat windowed scan + rebalanced engines.

out = (x - mean) / (sqrt(max(var,1e-6)) + 1e-6), 15x15 zero-padded box.
1 image (3,1024,1024) per NeuronCore, 9 stripes of <=114 output rows.

Per stripe:
  buf f32 = [15 zeros | x (DMA) | 7 zeros | 7x0.25 | sq | 7x0.25]
  sq   = Square(x - 0.5) f32            [ACT]
  o    = windowed 15-sum scan over buf  [DVE, fp16 out; x part = S1~,
         sq part = S2~ - 7.5 (constant folded into rsqrt bias)]
  PD   = -S1~ (band matmul on o1 view) then += 225*x (f32 identity mm)
  s1sq = Square(-PD + corr) fp16        [ACT, mid-group PSUM read]
  P2   = 225*(S2~-7.5) - s1sq           [band mm on o2 view + negI mm]
  t    = Identity(PD + d_scal) fp16     [ACT]
  R    = AbsRsqrt(P2 + corr') fp16      [ACT]
  outb = t * R -> f32                   [GPSIMD tensor_tensor]
DMA: input on sync/scalar HWDGE rings (alternating), output on the
opposite ring.
"""

import numpy as np
import ml_dtypes

C, H, W = 3, 1024, 1024
NCORES = 8
KS = 15
HALF = 7
MSTR = 114
NHALF = 512

PADL = 15
XD0 = PADL                  # 15
XD1 = XD0 + W               # 1039
G2 = XD1 + 7 + 7            # 1053 (7 zeros, 7 quarter pads)
SD0 = G2                    # 1053 sq data start
SD1 = SD0 + W               # 2077
BW2 = SD1 + 7               # 2084
SCN = BW2 - KS              # 2069 scan output length
O1OFS = HALF                # o index of x-box col j is j+7
O2OFS = SD0 + HALF - KS     # 1045

_CACHE = {}


def _stripes():
    out = []
    r_out = 0
    while r_out < H:
        m = min(MSTR, H - r_out)
        r_in0 = max(r_out - HALF, 0)
        r_in1 = min(r_out + m - 1 + HALF, H - 1)
        k = r_in1 - r_in0 + 1
        k_ofs = HALF - (r_out - r_in0)
        out.append((r_in0, k, r_out, m, k_ofs))
        r_out += m
    return out


def _const_mats():
    band = np.zeros((128, MSTR), dtype=np.float32)
    iden = np.zeros((128, MSTR), dtype=np.float32)
    for m in range(MSTR):
        band[m : m + KS, m] = 1.0
        iden[m + HALF, m] = 225.0
    band_top = np.zeros_like(band)
    band_top[0:121, :] = band[7:128, :]
    iden_top = np.zeros_like(iden)
    iden_top[0:121, :] = iden[7:128, :]
    negi = np.zeros((128, MSTR), dtype=np.float32)
    for m in range(MSTR):
        negi[m, m] = -1.0
    # fp16 stationaries: mid {-b, 225b}, top {-b, 225b}, negI
    mats16 = [-band, 225.0 * band, -band_top, 225.0 * band_top, negi]
    bands = np.stack(mats16, axis=1).astype(np.float16)  # [128, 5, 114]
    idens = np.stack([iden, iden_top], axis=1)           # [128, 2, 114] f32

    m_idx = np.arange(128)
    n_top = np.maximum(0, HALF - m_idx).astype(np.float32)
    n_bot = np.maximum(0, m_idx - 104).astype(np.float32)  # bottom stripe M=112
    corr = np.zeros((128, 3, 3), dtype=np.float32)
    # o2 rows carry a -7.5 offset each; vband sums (15-n) real rows.
    corr[:, 2, 0] = 0.0
    corr[:, 2, 1] = 225.0 * 7.5 * 15.0
    corr[:, 2, 2] = -112.5
    for v, n in ((0, n_top), (1, n_bot)):
        corr[:, v, 0] = -7.5 * n                       # Square(-PD + b)
        corr[:, v, 1] = 843.75 * n + 1687.5 * (15.0 - n)  # rsqrt bias
        corr[:, v, 2] = 7.5 * n - 112.5                # t = PD + d
    return bands, idens, corr


def _build_nc():
    import concourse.bass as bass
    import concourse.bacc as bacc
    import concourse.tile as tile
    from concourse import mybir
    import bass_rust as _bass_rust
    from concourse.hw_specs import get_activation_tables

    f32 = mybir.dt.float32
    fp16 = mybir.dt.float16
    Alu = mybir.AluOpType
    Act = mybir.ActivationFunctionType

    class _LceBacc(bacc.Bacc):
        """Pin ACT tables to the single set holding Square+Identity+AbsRsqrt."""

        def insert_act_table_loads(self):
            tables = [
                (name, funcs if name == "abs_reciprocal_sqrt_and_small" else set())
                for name, funcs in get_activation_tables(self.m.arch).items()
            ]
            _bass_rust.insert_act_table_loads(self, tables)

    nc = _LceBacc(trn_type="TRN2", target_bir_lowering=False)
    x_d = nc.dram_tensor("x", [C, H, W], f32, kind="ExternalInput")
    bands_d = nc.dram_tensor("bands", [128, 5, MSTR], fp16, kind="ExternalInput")
    idens_d = nc.dram_tensor("idens", [128, 2, MSTR], f32, kind="ExternalInput")
    corr_d = nc.dram_tensor("corr", [128, 3, 3], f32, kind="ExternalInput")
    y_d = nc.dram_tensor("y", [C, H, W], f32, kind="ExternalOutput")

    stripes = _stripes()

    from contextlib import ExitStack

    with tile.TileContext(nc) as tc, ExitStack() as ctx:
        singles = ctx.enter_context(tc.tile_pool(name="singles", bufs=1))
        io_pool = ctx.enter_context(tc.tile_pool(name="io", bufs=1))
        s1sq_p = ctx.enter_context(tc.tile_pool(name="s1sq", bufs=4))
        t_p = ctx.enter_context(tc.tile_pool(name="tt", bufs=4))
        r_p = ctx.enter_context(tc.tile_pool(name="rts", bufs=4))
        out_p = ctx.enter_context(tc.tile_pool(name="outb", bufs=4))
        psd_p = ctx.enter_context(tc.tile_pool(name="psd", bufs=2, space="PSUM"))
        ps2_p = ctx.enter_context(tc.tile_pool(name="ps2", bufs=2, space="PSUM"))

        bands_t = singles.tile([128, 5, MSTR], fp16)
        idens_t = singles.tile([128, 2, MSTR], f32)
        corr_t = singles.tile([128, 3, 3], f32)
        nc.sync.dma_start(out=bands_t[:, :, :], in_=bands_d[:, :, :])
        nc.sync.dma_start(out=idens_t[:, :, :], in_=idens_d[:, :, :])
        nc.sync.dma_start(out=corr_t[:, :, :], in_=corr_d[:, :, :])

        NBUF = 5
        xb = [io_pool.tile([128, BW2], f32, tag=f"xb{i}", name=f"xb{i}") for i in range(NBUF)]
        ob = [io_pool.tile([128, SCN], fp16, tag=f"ob{i}", name=f"ob{i}") for i in range(NBUF)]
        for i in range(NBUF):
            nc.vector.memset(xb[i][:, 0:XD0], 0.0)
            nc.vector.memset(xb[i][:, XD1 : XD1 + 7], 0.0)
            nc.vector.memset(xb[i][:, XD1 + 7 : G2], 0.25)
            nc.vector.memset(xb[i][:, SD1:BW2], 0.25)

        # ACT warm-ups: absorb const-DMA / memset sync ticks.
        neghalf = singles.tile([128, 1], f32)
        nc.vector.memset(neghalf[:, :], -0.5)
        warm1 = singles.tile([128, 1], f32)
        warm2 = singles.tile([128, 1], f32)
        warm3 = singles.tile([128, 1], f32)
        nc.scalar.activation(out=warm1[:, :], in_=corr_t[:, 0, 0:1], func=Act.Square)
        nc.scalar.activation(out=warm2[:, :], in_=neghalf[:, :], func=Act.Square)
        nc.scalar.activation(
            out=warm3[:, :], in_=warm2[:, :], func=Act.Abs_reciprocal_sqrt
        )

        it = 0
        for c in range(C):
            for r_in0, K, r_out0, M, k_ofs in stripes:
                i3 = it % NBUF
                xt, ot = xb[i3], ob[i3]

                dma_in = nc.sync.dma_start if it % 2 == 0 else nc.scalar.dma_start
                dma_out = nc.scalar.dma_start if it % 2 == 0 else nc.sync.dma_start
                it += 1

                dma_in(
                    out=xt[0:K, XD0:XD1],
                    in_=x_d[c, r_in0 : r_in0 + K, :],
                )

                # sq = (x - 0.5)^2, f32, into the concat buffer
                nc.scalar.activation(
                    out=xt[0:K, SD0:SD1],
                    in_=xt[0:K, XD0:XD1],
                    func=Act.Square,
                    bias=neghalf[0:K, 0:1],
                )

                # one windowed 15-sum scan over the whole buffer
                nc.vector.tensor_tensor_scan(
                    out=ot[0:K, 0:SCN],
                    data0=xt[0:K, KS:BW2],
                    data1=xt[0:K, 0:SCN],
                    initial=-7.5,
                    op0=Alu.add,
                    op1=Alu.subtract,
                )

                bofs = 2 if k_ofs else 0
                isel = 1 if k_ofs else 0
                vv = 0 if k_ofs else (1 if r_out0 + M == H else 2)
                sq_bias = corr_t[0:M, vv, 0:1]
                p2_bias = corr_t[0:M, vv, 1:2]
                d_scal = corr_t[0:M, vv, 2:3]

                pd = psd_p.tile([MSTR, W], f32)
                p2 = ps2_p.tile([MSTR, W], f32)
                # PD phase 1: PD = -S1~
                for j0 in (0, NHALF):
                    nc.tensor.matmul(
                        pd[0:M, j0 : j0 + NHALF],
                        bands_t[0:K, bofs, 0:M],
                        ot[0:K, O1OFS + j0 : O1OFS + j0 + NHALF],
                        start=True,
                        stop=False,
                    )
                # s1sq = (S1~)^2 = (-PD + corr)^2, fp16 (mid-group read)
                s1sq = s1sq_p.tile([MSTR, W], fp16)
                nc.scalar.activation(
                    out=s1sq[0:M, :],
                    in_=pd[0:M, :],
                    func=Act.Square,
                    scale=-1.0,
                    bias=sq_bias,
                )
                for j0 in (0, NHALF):
                    # PD phase 2: += 225*x  (f32 identity matmul)
                    nc.tensor.matmul(
                        pd[0:M, j0 : j0 + NHALF],
                        idens_t[0:K, isel, 0:M],
                        xt[0:K, XD0 + j0 : XD0 + j0 + NHALF],
                        start=False,
                        stop=True,
                        skip_group_check=True,
                    )
                    # P2 = 225*(S2~ - 7.5) - s1sq
                    nc.tensor.matmul(
                        p2[0:M, j0 : j0 + NHALF],
                        bands_t[0:K, bofs + 1, 0:M],
                        ot[0:K, O2OFS + j0 : O2OFS + j0 + NHALF],
                        start=True,
                        stop=False,
                    )
                    nc.tensor.matmul(
                        p2[0:M, j0 : j0 + NHALF],
                        bands_t[0:M, 4, 0:M],
                        s1sq[0:M, j0 : j0 + NHALF],
                        start=False,
                        stop=True,
                    )
                # t = PD + d_scal (fp16) -- frees pd before rsqrt
                tt = t_p.tile([MSTR, W], fp16)
                nc.scalar.activation(
                    out=tt[0:M, :],
                    in_=pd[0:M, :],
                    func=Act.Identity,
                    bias=d_scal,
                )
                # R = rsqrt(P2 + bias) fp16
                rts = r_p.tile([MSTR, W], fp16)
                nc.scalar.activation(
                    out=rts[0:M, :],
                    in_=p2[0:M, :],
                    func=Act.Abs_reciprocal_sqrt,
                    bias=p2_bias,
                )
                # out = t * R  (GPSIMD)
                outb = out_p.tile([MSTR, W], f32)
                nc.gpsimd.tensor_tensor(
                    outb[0:M, :], tt[0:M, :], rts[0:M, :], Alu.mult
                )
                dma_out(out=y_d[c, r_out0 : r_out0 + M, :], in_=outb[0:M, :])

    nc.finalize()
    return nc


def _get_nc():
    if "nc" not in _CACHE:
        _CACHE["nc"] = _build_nc()
    return _CACHE["nc"]


def kernel(x: np.ndarray, _trace: bool = False, _tmpdir=None) -> np.ndarray:
    from concourse.bass_utils import run_bass_kernel_spmd

    assert x.shape == (NCORES, C, H, W), x.shape
    nc = _get_nc()
    bands, idens, corr = _const_mats()
    in_maps = [
        {
            "x": np.ascontiguousarray(x[i]).astype(np.float32, copy=False),
            "bands": bands,
            "idens": idens,
            "corr": corr,
        }
        for i in range(NCORES)
    ]
    res = run_bass_kernel_spmd(
        nc,
        in_maps,
        core_ids=list(range(NCORES)),
        trace=_trace,
        tmpdir=_tmpdir,
    )
    _CACHE["last_results"] = res
    out = np.stack([r["y"] for r in res.results], axis=0)
    return out


if __name__ == "__main__":
    rng = np.random.default_rng(0)
    x = rng.random((NCORES, C, H, W), dtype=np.float32)
    y = kernel(x)
    print(y.shape, y.dtype, float(np.abs(y).mean()))


# revision 5
# speedup vs baseline: 1.0580x; 1.0580x over previous
"""LocalContrastEnhancement v5: concat windowed scan + software-pipelined emission.

out = (x - mean) / (sqrt(max(var,1e-6)) + 1e-6), 15x15 zero-padded box.
1 image (3,1024,1024) per NeuronCore, 9 stripes of <=114 output rows.

Math per stripe (see v4):
  buf f32 = [15 zeros | x (DMA) | 7 zeros | 7x0.25 | sq | 7x0.25]
  sq   = Square(x - 0.5) f32                 [ACT]
  o    = windowed 15-sum scan over buf, fp16 [DVE]  (x part = S1~,
         sq part = S2~ - 7.5; constant folded into the rsqrt bias)
  PD   = -S1~ (fp16 band mm on o1 view) += 225*x (f32 identity mm)
  s1sq = Square(-PD + corr) fp16             [ACT, mid-group PSUM read]
  P2   = 225*(S2~-7.5) - s1sq                [fp16 band mm on o2 + negI mm]
  t    = Identity(PD + d_scal) fp16          [ACT]
  R    = AbsRsqrt(P2 + corr') fp16           [ACT]
  outb = t * R -> f32                        [GPSIMD tensor_tensor]

Emission is skewed so no engine FIFO head blocks ready work:
  iteration i emits  dma_in(i+2) | sq(i+1), scan(i+1) | PE/s1sq/t/rsqrt/
  gp/dma_out(i).  ACT order per iteration: sq(i+1), s1sq(i), t(i), rsqrt(i).
Inputs alternate the sync/scalar HWDGE rings; outputs take the other ring.
A prologue matmul burst holds the PE HAM clock-gate at 2.4 GHz.
"""

import numpy as np
import ml_dtypes

C, H, W = 3, 1024, 1024
NCORES = 8
KS = 15
HALF = 7
MSTR = 114
NHALF = 512

PADL = 15
XD0 = PADL                  # 15
XD1 = XD0 + W               # 1039
G2 = XD1 + 7 + 7            # 1053
SD0 = G2                    # 1053 sq data start
SD1 = SD0 + W               # 2077
BW2 = SD1 + 7               # 2084
SCN = BW2 - KS              # 2069 scan output length
O1OFS = HALF                # o index of x-box col j is j+7
O2OFS = SD0 + HALF - KS     # 1045

_CACHE = {}


def _stripes():
    out = []
    r_out = 0
    while r_out < H:
        m = min(MSTR, H - r_out)
        r_in0 = max(r_out - HALF, 0)
        r_in1 = min(r_out + m - 1 + HALF, H - 1)
        k = r_in1 - r_in0 + 1
        k_ofs = HALF - (r_out - r_in0)
        out.append((r_in0, k, r_out, m, k_ofs))
        r_out += m
    return out


def _const_mats():
    band = np.zeros((128, MSTR), dtype=np.float32)
    iden = np.zeros((128, MSTR), dtype=np.float32)
    for m in range(MSTR):
        band[m : m + KS, m] = 1.0
        iden[m + HALF, m] = 225.0
    band_top = np.zeros_like(band)
    band_top[0:121, :] = band[7:128, :]
    iden_top = np.zeros_like(iden)
    iden_top[0:121, :] = iden[7:128, :]
    negi = np.zeros((128, MSTR), dtype=np.float32)
    for m in range(MSTR):
        negi[m, m] = -1.0
    mats16 = [-band, 225.0 * band, -band_top, 225.0 * band_top, negi]
    bands = np.stack(mats16, axis=1).astype(np.float16)  # [128, 5, 114]
    idens = np.stack([iden, iden_top], axis=1)           # [128, 2, 114] f32

    m_idx = np.arange(128)
    n_top = np.maximum(0, HALF - m_idx).astype(np.float32)
    n_bot = np.maximum(0, m_idx - 104).astype(np.float32)  # bottom stripe M=112
    corr = np.zeros((128, 3, 3), dtype=np.float32)
    corr[:, 2, 0] = 0.0
    corr[:, 2, 1] = 225.0 * 7.5 * 15.0
    corr[:, 2, 2] = -112.5
    for v, n in ((0, n_top), (1, n_bot)):
        corr[:, v, 0] = -7.5 * n
        corr[:, v, 1] = 843.75 * n + 1687.5 * (15.0 - n)
        corr[:, v, 2] = 7.5 * n - 112.5
    return bands, idens, corr


def _build_nc():
    import concourse.bass as bass
    import concourse.bacc as bacc
    import concourse.tile as tile
    from concourse import mybir
    import bass_rust as _bass_rust
    from concourse.hw_specs import get_activation_tables

    f32 = mybir.dt.float32
    fp16 = mybir.dt.float16
    Alu = mybir.AluOpType
    Act = mybir.ActivationFunctionType

    class _LceBacc(bacc.Bacc):
        """Pin ACT tables to the single set holding Square+Identity+AbsRsqrt."""

        def insert_act_table_loads(self):
            tables = [
                (name, funcs if name == "abs_reciprocal_sqrt_and_small" else set())
                for name, funcs in get_activation_tables(self.m.arch).items()
            ]
            _bass_rust.insert_act_table_loads(self, tables)

    nc = _LceBacc(trn_type="TRN2", target_bir_lowering=False)
    x_d = nc.dram_tensor("x", [C, H, W], f32, kind="ExternalInput")
    bands_d = nc.dram_tensor("bands", [128, 5, MSTR], fp16, kind="ExternalInput")
    idens_d = nc.dram_tensor("idens", [128, 2, MSTR], f32, kind="ExternalInput")
    corr_d = nc.dram_tensor("corr", [128, 3, 3], f32, kind="ExternalInput")
    y_d = nc.dram_tensor("y", [C, H, W], f32, kind="ExternalOutput")

    stripes = _stripes()
    iters = []
    for c in range(C):
        for r_in0, K, r_out0, M, k_ofs in stripes:
            iters.append((c, r_in0, K, r_out0, M, k_ofs))
    NST = len(iters)

    from contextlib import ExitStack

    with tile.TileContext(nc) as tc, ExitStack() as ctx:
        singles = ctx.enter_context(tc.tile_pool(name="singles", bufs=1))
        io_pool = ctx.enter_context(tc.tile_pool(name="io", bufs=1))
        s1sq_p = ctx.enter_context(tc.tile_pool(name="s1sq", bufs=4))
        t_p = ctx.enter_context(tc.tile_pool(name="tt", bufs=4))
        r_p = ctx.enter_context(tc.tile_pool(name="rts", bufs=4))
        out_p = ctx.enter_context(tc.tile_pool(name="outb", bufs=4))
        psd_p = ctx.enter_context(tc.tile_pool(name="psd", bufs=2, space="PSUM"))
        ps2_p = ctx.enter_context(tc.tile_pool(name="ps2", bufs=2, space="PSUM"))

        bands_t = singles.tile([128, 5, MSTR], fp16)
        idens_t = singles.tile([128, 2, MSTR], f32)
        corr_t = singles.tile([128, 3, 3], f32)
        nc.sync.dma_start(out=bands_t[:, :, :], in_=bands_d[:, :, :])
        nc.sync.dma_start(out=idens_t[:, :, :], in_=idens_d[:, :, :])
        nc.sync.dma_start(out=corr_t[:, :, :], in_=corr_d[:, :, :])

        NBUF = 5
        xb = [io_pool.tile([128, BW2], f32, tag=f"xb{i}", name=f"xb{i}") for i in range(NBUF)]
        ob = [io_pool.tile([128, SCN], fp16, tag=f"ob{i}", name=f"ob{i}") for i in range(NBUF)]
        for i in range(NBUF):
            nc.vector.memset(xb[i][:, 0:XD0], 0.0)
            nc.vector.memset(xb[i][:, XD1 : XD1 + 7], 0.0)
            nc.vector.memset(xb[i][:, XD1 + 7 : G2], 0.25)
            nc.vector.memset(xb[i][:, SD1:BW2], 0.25)

        neghalf = singles.tile([128, 1], f32)
        nc.vector.memset(neghalf[:, :], -0.5)
        warm1 = singles.tile([128, 1], f32)
        warm2 = singles.tile([128, 1], f32)
        warm3 = singles.tile([128, 1], f32)
        nc.scalar.activation(out=warm1[:, :], in_=corr_t[:, 0, 0:1], func=Act.Square)
        nc.scalar.activation(out=warm2[:, :], in_=neghalf[:, :], func=Act.Square)
        nc.scalar.activation(
            out=warm3[:, :], in_=warm2[:, :], func=Act.Abs_reciprocal_sqrt
        )
        # PE warm-up burst: sustained matmul activity flips the HAM clock
        # gate to 8/8 before stripe 0's matmuls arrive.
        pwarm = psd_p.tile([MSTR, W], f32, tag="pd")
        for _ in range(28):
            nc.tensor.matmul(
                pwarm[0:MSTR, 0:MSTR],
                bands_t[0:128, 1, 0:MSTR],
                bands_t[0:128, 0, 0:MSTR],
                start=True,
                stop=True,
            )
        warm4 = singles.tile([128, 1], f32)
        nc.scalar.activation(out=warm4[0:1, :], in_=pwarm[0:1, 0:1], func=Act.Copy)

        def dma_in(it):
            c, r_in0, K, r_out0, M, k_ofs = iters[it]
            eng = nc.sync if it % 2 == 0 else nc.scalar
            eng.dma_start(
                out=xb[it % NBUF][0:K, XD0:XD1],
                in_=x_d[c, r_in0 : r_in0 + K, :],
            )

        def front(it):
            """square + scan for stripe it"""
            c, r_in0, K, r_out0, M, k_ofs = iters[it]
            xt, ot = xb[it % NBUF], ob[it % NBUF]
            nc.scalar.activation(
                out=xt[0:K, SD0:SD1],
                in_=xt[0:K, XD0:XD1],
                func=Act.Square,
                bias=neghalf[0:K, 0:1],
            )
            nc.vector.tensor_tensor_scan(
                out=ot[0:K, 0:SCN],
                data0=xt[0:K, KS:BW2],
                data1=xt[0:K, 0:SCN],
                initial=-7.5,
                op0=Alu.add,
                op1=Alu.subtract,
            )

        def back(it):
            """matmuls, s1sq/t/rsqrt, gp mult, out-dma for stripe it"""
            c, r_in0, K, r_out0, M, k_ofs = iters[it]
            xt, ot = xb[it % NBUF], ob[it % NBUF]
            bofs = 2 if k_ofs else 0
            isel = 1 if k_ofs else 0
            vv = 0 if k_ofs else (1 if r_out0 + M == H else 2)
            sq_bias = corr_t[0:M, vv, 0:1]
            p2_bias = corr_t[0:M, vv, 1:2]
            d_scal = corr_t[0:M, vv, 2:3]

            pd = psd_p.tile([MSTR, W], f32, tag="pd")
            p2 = ps2_p.tile([MSTR, W], f32)
            for j0 in (0, NHALF):
                nc.tensor.matmul(
                    pd[0:M, j0 : j0 + NHALF],
                    bands_t[0:K, bofs, 0:M],
                    ot[0:K, O1OFS + j0 : O1OFS + j0 + NHALF],
                    start=True,
                    stop=False,
                )
            s1sq = s1sq_p.tile([MSTR, W], fp16)
            nc.scalar.activation(
                out=s1sq[0:M, :],
                in_=pd[0:M, :],
                func=Act.Square,
                scale=-1.0,
                bias=sq_bias,
            )
            for j0 in (0, NHALF):
                nc.tensor.matmul(
                    pd[0:M, j0 : j0 + NHALF],
                    idens_t[0:K, isel, 0:M],
                    xt[0:K, XD0 + j0 : XD0 + j0 + NHALF],
                    start=False,
                    stop=True,
                    skip_group_check=True,
                )
                nc.tensor.matmul(
                    p2[0:M, j0 : j0 + NHALF],
                    bands_t[0:K, bofs + 1, 0:M],
                    ot[0:K, O2OFS + j0 : O2OFS + j0 + NHALF],
                    start=True,
                    stop=False,
                )
            for j0 in (0, NHALF):
                nc.tensor.matmul(
                    p2[0:M, j0 : j0 + NHALF],
                    bands_t[0:M, 4, 0:M],
                    s1sq[0:M, j0 : j0 + NHALF],
                    start=False,
                    stop=True,
                )
            tt = t_p.tile([MSTR, W], fp16)
            nc.scalar.activation(
                out=tt[0:M, :],
                in_=pd[0:M, :],
                func=Act.Identity,
                bias=d_scal,
            )
            rts = r_p.tile([MSTR, W], fp16)
            nc.scalar.activation(
                out=rts[0:M, :],
                in_=p2[0:M, :],
                func=Act.Abs_reciprocal_sqrt,
                bias=p2_bias,
            )
            outb = out_p.tile([MSTR, W], f32)
            nc.gpsimd.tensor_tensor(
                outb[0:M, :], tt[0:M, :], rts[0:M, :], Alu.mult
            )
            eng = nc.scalar if it % 2 == 0 else nc.sync
            eng.dma_start(out=y_d[c, r_out0 : r_out0 + M, :], in_=outb[0:M, :])

        # skewed pipeline
        dma_in(0)
        dma_in(1)
        front(0)
        for it in range(NST):
            if it + 2 < NST:
                dma_in(it + 2)
            if it + 1 < NST:
                front(it + 1)
            back(it)

    nc.finalize()
    return nc


def _get_nc():
    if "nc" not in _CACHE:
        _CACHE["nc"] = _build_nc()
    return _CACHE["nc"]


def kernel(x: np.ndarray, _trace: bool = False, _tmpdir=None) -> np.ndarray:
    from concourse.bass_utils import run_bass_kernel_spmd

    assert x.shape == (NCORES, C, H, W), x.shape
    nc = _get_nc()
    bands, idens, corr = _const_mats()
    in_maps = [
        {
            "x": np.ascontiguousarray(x[i]).astype(np.float32, copy=False),
            "bands": bands,
            "idens": idens,
            "corr": corr,
        }
        for i in range(NCORES)
    ]
    res = run_bass_kernel_spmd(
        nc,
        in_maps,
        core_ids=list(range(NCORES)),
        trace=_trace,
        tmpdir=_tmpdir,
    )
    _CACHE["last_results"] = res
    out = np.stack([r["y"] for r in res.results], axis=0)
    return out


if __name__ == "__main__":
    rng = np.random.default_rng(0)
    x = rng.random((NCORES, C, H, W), dtype=np.float32)
    y = kernel(x)
    print(y.shape, y.dtype, float(np.abs(y).mean()))
